# revision 54
# baseline (speedup 1.0000x reference)
"""PointPillarsScatter on 8 TRN2 NeuronCores.

Reference op: scatter N pillar feature vectors [N, 64] into a canvas
[B=4, C=64, NY=496, NX=432] at (y, x) cell coords (zero elsewhere).

Sharding: 8 cores = 4 batches x 2 y-halves. Core k=(b, g) owns the
canvas slice out[b, :, 248*g : 248*(g+1), :] -> flat [64, 107136].

Device algorithm (per core), all standard engine ops:
  - canvas is produced in column-windows of W=512 cells across 2
    column-slabs stacked on partitions: window tile [128, 512] where
    partition p = 64*a + c (a = slab, c = channel).
  - for each window, host packs the <=K pillars that land in it (both
    slabs share one slot space) into lhsT weights [K slots, 128] with
    w[k, 64*slab_k + c] = feat[pillar_k, c] (fp16), plus a local column
    index idx[k] in [0, 512).
  - DVE builds onehot[k, j] = (iota[j] == idx[k]) in fp16 with one
    tensor_scalar (fp16 packed operands hit the DVE fast path; iota is
    generated on-device; the idx scalar stays f32 as is_equal requires).
  - PE matmul lhsT.T @ onehot -> PSUM f32 [128, 512] = the scattered
    window (indices <= 511 are exact in fp16 so the compare is exact;
    onehot rows are 0/1 so occupied cells get the exact fp16 feature).
  - PSUM -> SBUF copies cast to fp16 at copy_gran=2-window granularity,
    mostly on ACT with 1-in-6 on DVE ("AAAAAD"): paired HW measurement
    ranks AAAAAD (~49 us/iter pooled) < all-ACT (54.9) < the cost
    model's preferred balanced ACT/DVE split (60.4); 13 DVE copies
    collapse to 77.5 — DVE PSUM-read copies respond sharply and
    non-linearly on real HW, so a light sprinkle is the optimum.
    GPSIMD cannot read PSUM, and its software tensor ops measure ~9x
    the cost model on real HW — keep GPSIMD to SWDGE weight-DMA
    descriptor generation only.
  - accumulate SUPER=4 windows into one [128, 2048] fp16 tile, DMA to
    a CONTIGUOUS DRAM superblock (scattered multi-descriptor DMA is
    far below line rate; contiguous superblocks run at full rate).
  - ramp-up: small head weight-groups ride the idle SP queue and dummy
    matmuls warm the PE p-state while the idx DMA stages.
  - host unscrambles superblocks into the final canvas layout and
    upcasts fp16 -> f32 (abs err ~5e-4 * |x|, far under the 2e-2 gate).

Measured: paired repeat-unroll HW slope, two independent R=200 runs =
52.6 and 44.1 us/iter (pooled 24-pair median 49.3 us), vs 50.8 us for
the pure-DMA skeleton by the same method — i.e. at the hardware DMA
floor within measurement noise (numbers include the ~15% unroll
artifact; single-shot is faster).  Cost model: 63.5 us here, but
the model over-costs ACT-copy chains and under-costs DVE copies — HW
measurements, not the model, chose this copy plan.  Baseline for
comparison: 117.3 us harness HW / 111.1 us model.

Self-contained: shapes hardcoded, no sibling imports.
"""

import numpy as np

NY, NX, C = 496, 432, 64
B = 4
N_CORES = 8
HALF_Y = NY // 2  # 248
CORE_COLS = HALF_Y * NX  # 107136 canvas cells per core
SLABS = 2
SLAB = CORE_COLS // SLABS  # 53568
W = 512  # window width (canvas cells per matmul)
NWIN = (SLAB + W - 1) // W  # 105 windows (last = 320 cols)
LAST_W = SLAB - (NWIN - 1) * W  # 320
K_SLOTS = 80  # pillar slots per window chunk (both slabs share them)
GROUP = 16  # weight-tile entries fetched per input DMA
SUPER = 4  # windows per output superblock DMA
NSB = NWIN // SUPER  # full superblocks; remainder windows after that
REM_WINS = NWIN - NSB * SUPER
OUT_ELEMS = C * CORE_COLS  # per-core output element count

_cache = {}


def _copy_plan(chunks_per_window, nwt):
    """Greedy engine assignment for the per-window PSUM->SBUF copies.

    Engine-busy costs from the TRN2 cost model (ns): DVE carries the
    onehot builds (~194 each) plus any copies (658); ACT copy 612.
    GPSIMD/Pool cannot read PSUM (neuronxcc rejects it), so only A/D.
    Assign each window's copy to the engine with the least accumulated
    busy time.
    """
    loads = {"D": 194.0 * nwt, "A": 0.0}
    cost = {"D": 658.0, "A": 612.0}
    plan = []
    for _ in range(NWIN):
        eng = min("AD", key=lambda e: loads[e] + cost[e])
        loads[eng] += cost[eng]
        plan.append(eng)
    return "".join(plan)


def _copy_plan_gran(nwt, gran, dve_oh_frac=1.0, super_w=SUPER,
                    pin_final=False):
    """D/A assignment when one copy covers `gran` windows.

    The FINAL copy of each superblock is pinned to ACT so the scalar
    engine can issue the superblock's out-DMA with its sem already
    satisfied (wait-free issue).  Earlier copies in the superblock go
    greedily to the less-loaded engine.
    """
    cost = {"D": gran * 512 * 1.0417 + 125.0, "A": gran * 512 * 0.8333 + 185.0}
    per_sb = super_w // gran
    nsb = (NWIN - REM_WINS) // super_w
    ncopies = nsb * per_sb + REM_WINS
    # DVE's onehot work accrues progressively with the window stream, so
    # charge it per copy slot rather than up front — otherwise the plan
    # front-loads every ACT copy and the early superblocks serialize on
    # one engine.
    oh_per_copy = 194.0 * nwt * dve_oh_frac / ncopies
    loads = {"D": 0.0, "A": 0.0}
    plan = []
    for s in range(nsb):
        for i in range(per_sb):
            loads["D"] += oh_per_copy
            pin = (pin_final == "all"
                   or (pin_final == "odd" and s % 2 == 1))
            if i == per_sb - 1 and pin:
                eng = "A"
            else:
                eng = min("AD", key=lambda e: loads[e] + cost[e])
            loads[eng] += cost[eng]
            plan.append(eng)
    plan.extend("A" * REM_WINS)
    return "".join(plan)


def _build_program(chunks_per_window, nwt, repeat=1, copy_plan="AAAAAD",
                   psum_bufs=4, oh_bufs=8, sb_bufs=6, wt_bufs=5,
                   super_w=SUPER, group=GROUP, mode="full", copy_gran=2,
                   w_dma_eng="gpsimd", oh_plan="D", out_dma_mode="sp",
                   head_groups=(2, 6), warm_mm=6, split_first=2):
    """Build the shared SPMD bass program for the given window schedule.

    chunks_per_window: list[int] of length NWIN (>=1 each), shared by all
    cores. nwt == sum(chunks_per_window) weight-tile entries.
    mode: "full" | "dmaonly" (no compute, DMA constant tiles) | "nodma"
    (compute, tiny out-DMA) | "noonehot" (one shared onehot) | "nocopy"
    (PSUM never copied; DMA a memset tile) — timing bisection only.
    """
    import concourse.bacc as bacc
    import concourse.bass as bass
    import concourse.tile as tile
    import concourse.mybir as mybir
    from contextlib import ExitStack

    f32 = mybir.dt.float32
    f16 = mybir.dt.float16

    if copy_plan is None:
        dve_frac = oh_plan.count("D") / len(oh_plan)
        pin = {"copyeng": "all", "mixed": "odd"}.get(out_dma_mode, "no")
        copy_plan = (_copy_plan(chunks_per_window, nwt) if copy_gran == 1
                     else _copy_plan_gran(nwt, copy_gran, dve_frac, super_w,
                                          pin))

    nc = bacc.Bacc("TRN2", target_bir_lowering=False, debug=False,
                   num_devices=N_CORES)

    w_dram = nc.dram_tensor("w", [K_SLOTS, nwt * 128], f16,
                            kind="ExternalInput")
    idx_dram = nc.dram_tensor("idx", [K_SLOTS, nwt], f32,
                              kind="ExternalInput")
    # scrambled output: NSB superblocks [128, SUPER*W] + remainder windows
    out_dram = nc.dram_tensor("out", [1, OUT_ELEMS], f16,
                              kind="ExternalOutput")

    SUP = super_w
    NSB_L = NWIN // SUP
    with tile.TileContext(nc) as tc, ExitStack() as ctx:
        const_pool = ctx.enter_context(tc.tile_pool(name="const", bufs=1))
        w_pool = ctx.enter_context(tc.tile_pool(name="wpool", bufs=wt_bufs))
        oh_pool = ctx.enter_context(tc.tile_pool(name="ohpool", bufs=oh_bufs))
        out_pool = ctx.enter_context(tc.tile_pool(name="opool", bufs=sb_bufs))
        psum_pool = ctx.enter_context(
            tc.tile_pool(name="pspool", bufs=psum_bufs, space="PSUM"))

        # iota generated on-device (integers < 2048 are exact in fp16), so
        # the ramp-up chain only waits on the idx DMA
        iota_t = const_pool.tile([K_SLOTS, W], f16)
        nc.gpsimd.iota(iota_t[:], [[1, W]], base=0, channel_multiplier=0,
                       allow_small_or_imprecise_dtypes=True)
        idx_t = const_pool.tile([K_SLOTS, nwt], f32)
        nc.sync.dma_start(idx_t[:], idx_dram.ap())
        shared_oh = None
        if mode == "noonehot":
            shared_oh = const_pool.tile([K_SLOTS, W], f16)
            nc.vector.tensor_scalar(
                shared_oh[:], iota_t[:], idx_t[:, 0:1], None,
                op0=mybir.AluOpType.is_equal)
        zed = None
        if mode in ("dmaonly", "nocopy"):
            zed = const_pool.tile([128, SUP * W], f16)
            nc.vector.memset(zed[:], 0.125)

        if warm_mm and mode not in ("dmaonly",):
            # PE p-state warm-up: the tensor engine runs at 0.65/1.2 GHz
            # until it has ~3us of continuous execution.  Burn that in on a
            # scratch PSUM bank while the const/weight DMAs stage, so the
            # first real matmuls run at the full 2.4 GHz.
            wsrc = const_pool.tile([1, W], f16)
            nc.vector.memset(wsrc[:], 0.0)
            wdst = psum_pool.tile([128, W], f32, tag="ps", name="warm")
            for i in range(warm_mm):
                nc.tensor.matmul(wdst[:], wsrc[:, :128], wsrc[:],
                                 start=True, stop=True)

        G = copy_gran
        assert SUP % G == 0
        # weight-group schedule: small head groups shorten the ramp-up
        # critical chain (first matmul can start after a 2-entry fetch)
        gsizes = [g for g in head_groups if g > 0]
        covered = sum(gsizes)
        if covered >= nwt:
            gsizes, covered = [], 0
        while covered < nwt:
            gsizes.append(min(group, nwt - covered))
            covered += gsizes[-1]
        gstart = [0]
        for gs in gsizes:
            gstart.append(gstart[-1] + gs)
        entry_group = []
        for gi, gs in enumerate(gsizes):
            entry_group.extend([gi] * gs)
        n_head = len([g for g in head_groups if g > 0])
        for rep in range(repeat):
            e = 0
            w_tiles = {}
            sb_tile = None
            sb_base = 0  # first window index of current superblock
            ps_group = None
            ps_base = 0
            copy_i = 0
            last_cp_eng = nc.sync
            for w in range(NWIN):
                n = W if w < NWIN - 1 else LAST_W
                in_super = w < NSB_L * SUP
                if in_super and w % SUP == 0:
                    sb_tile = out_pool.tile([128, SUP * W], f16, tag="sb",
                                            name=f"sb_{rep}_{w // SUP}")
                    sb_base = w
                nchunks = chunks_per_window[w] if mode != "dmaonly" else 0
                if in_super:
                    if w % G == 0:
                        ps_group = psum_pool.tile([128, G * W], f32,
                                                  tag="ps",
                                                  name=f"ps_{rep}_{w // G}")
                        ps_base = w
                    ps = ps_group[:, (w - ps_base) * W : (w - ps_base + 1) * W]
                else:
                    ps_group = psum_pool.tile([128, W], f32, tag="ps",
                                              name=f"ps_{rep}_r{w}")
                    ps_base = w
                    ps = ps_group[:, :W]
                for t in range(nchunks):
                    g = entry_group[e]
                    if g not in w_tiles:
                        glen = gsizes[g]
                        wt = w_pool.tile([K_SLOTS, glen * 128], f16,
                                         tag="wt", name=f"wt_{rep}_{g}")
                        # dense block-diagonal weights straight from DRAM:
                        # per partition one contiguous glen*128 fp16 run
                        # so the DMA runs at full descriptor rate.
                        dst = bass.AP(wt.tensor, wt.offset,
                                      [[glen * 128, K_SLOTS],
                                       [1, glen * 128]])
                        src = bass.AP(w_dram, gstart[g] * 128,
                                      [[nwt * 128, K_SLOTS],
                                       [1, glen * 128]])
                        # weight loads go on their own engine queue so
                        # they never sit behind an out-DMA's sem-wait on
                        # the SP sequencer (head-of-line blocking); the
                        # head groups ride the idle SP queue for a fast
                        # ramp-up.
                        weng = nc.sync if g < n_head else getattr(
                            nc, w_dma_eng)
                        weng.dma_start(dst, src)
                        w_tiles[g] = wt
                    wt = w_tiles[g]
                    woff = (e - gstart[g]) * 128
                    if mode == "noonehot":
                        oh = shared_oh
                    else:
                        oh = oh_pool.tile([K_SLOTS, W], f16, tag="oh",
                                          name=f"oh_{rep}_{w}_{t}")
                        oh_eng = (nc.gpsimd
                                  if oh_plan[e % len(oh_plan)] == "P"
                                  else nc.vector)
                        oh_eng.tensor_scalar(
                            oh[:, :n], iota_t[:, :n], idx_t[:, e : e + 1],
                            None, op0=mybir.AluOpType.is_equal)
                    nc.tensor.matmul(
                        ps[:, :n], wt[:, woff : woff + 128], oh[:, :n],
                        start=(t == 0), stop=(t == nchunks - 1))
                    e += 1
                if in_super:
                    group_end = (w - ps_base) == G - 1
                    cp_src = ps_group[:] if group_end else None
                    j0 = (ps_base - sb_base) * W
                    cp_dst = sb_tile[:, j0 : j0 + G * W] if group_end else None
                else:
                    sb_tile = out_pool.tile([128, SUP * W], f16, tag="sb",
                                            name=f"sb_{rep}_r{w}")
                    cp_src = ps_group[:, :n]
                    cp_dst = sb_tile[:, :n]
                if mode not in ("dmaonly", "nocopy") and cp_dst is not None:
                    eng = copy_plan[copy_i % len(copy_plan)]
                    copy_i += 1
                    if eng == "D":
                        last_cp_eng = nc.vector
                        nc.vector.tensor_copy(cp_dst, cp_src)
                    elif eng == "P":
                        last_cp_eng = nc.gpsimd
                        nc.gpsimd.tensor_copy(cp_dst, cp_src)
                    else:
                        last_cp_eng = nc.scalar
                        nc.scalar.copy(cp_dst, cp_src)
                if mode == "nodma":
                    off = w * 128 * 16
                    dst = bass.AP(out_dram, off, [[16, 128], [1, 16]])
                    nc.sync.dma_start(dst, sb_tile[:, :16])
                    continue
                # out-DMA issued by the engine that produced the final copy:
                # its sem is already satisfied, so the descriptor-gen
                # overhead never serializes on a single waiting sequencer.
                # Only SP/Activation (HWDGE) and gpsimd (SWDGE) can issue.
                out_eng = nc.sync
                if out_dma_mode == "copyeng" and mode not in ("dmaonly",
                                                              "nocopy"):
                    if last_cp_eng is nc.scalar or last_cp_eng is nc.gpsimd:
                        out_eng = last_cp_eng
                elif out_dma_mode == "split":
                    out_eng = nc.sync if (w // SUP) % 2 == 0 else nc.gpsimd
                elif out_dma_mode == "mixed" and in_super:
                    out_eng = nc.sync if (w // SUP) % 2 == 0 else nc.scalar
                src_tile = sb_tile if mode not in ("dmaonly", "nocopy") else zed
                first_split = (rep == 0 and mode == "full"
                               and w // SUP < split_first)
                if in_super and first_split and (w - ps_base) == G - 1:
                    # ramp-up: the leading superblocks DMA per copy-group so
                    # the out stream starts ~2 windows earlier (same DRAM
                    # layout, strided rows still >= 512B -> full rate)
                    off = sb_base * 128 * W + j0
                    dst = bass.AP(out_dram, off, [[SUP * W, 128],
                                                  [1, G * W]])
                    out_eng.dma_start(dst, src_tile[:, j0 : j0 + G * W])
                elif in_super and not first_split and (w - sb_base) == SUP - 1:
                    off = sb_base * 128 * W
                    dst = bass.AP(out_dram, off, [[SUP * W, 128],
                                                  [1, SUP * W]])
                    out_eng.dma_start(dst, src_tile[:])
                elif not in_super:
                    off = NSB_L * SUP * 128 * W + (w - NSB_L * SUP) * 128 * LAST_W
                    dst = bass.AP(out_dram, off, [[n, 128], [1, n]])
                    out_eng.dma_start(dst, src_tile[:, :n])
            assert e == nwt or mode == "dmaonly"

    nc.compile()
    return nc


def _unscramble(core_flat, out_view):
    """[OUT_ELEMS] fp16 scrambled superblocks -> f32 canvas [C, HALF_Y, NX].

    out_view: the [C, HALF_Y, NX] f32 slice of the final output to fill.
    """
    canvas_v = out_view.reshape(C, SLABS, SLAB)
    main = core_flat[: NSB * 128 * SUPER * W].reshape(
        NSB, SLABS, C, SUPER * W)  # [g, a, c, j]
    # canvas cols a*SLAB + g*SUPER*W + j  for j in [0, SUPER*W)
    m = main.transpose(2, 1, 0, 3).reshape(C, SLABS, NSB * SUPER * W)
    canvas_v[:, :, : NSB * SUPER * W] = m  # upcasts fp16 -> f32
    off = NSB * 128 * SUPER * W
    for r in range(REM_WINS):
        w = NSB * SUPER + r
        blk = core_flat[off : off + 128 * LAST_W].reshape(SLABS, C, LAST_W)
        canvas_v[:, :, w * W : w * W + LAST_W] = blk.transpose(1, 0, 2)
        off += 128 * LAST_W


def _host_pack(voxel_features, coords):
    """Shard + pack inputs for the 8 cores.

    Returns (in_maps, chunks_per_window, nwt).
    """
    vf = np.asarray(voxel_features, dtype=np.float32)
    vf16 = vf.astype(np.float16)
    cd = np.asarray(coords)
    bidx = cd[:, 0].astype(np.int64)
    yy = cd[:, 2].astype(np.int64)
    xx = cd[:, 3].astype(np.int64)

    # jax scatter drops out-of-bounds indices; match by masking them out
    inb = (yy >= 0) & (yy < NY) & (xx >= 0) & (xx < NX)

    cores = []
    counts_per_core = []
    for b in range(B):
        for g in range(2):
            sel = np.nonzero(inb & (bidx == b) & (yy >= g * HALF_Y)
                             & (yy < (g + 1) * HALF_Y))[0]
            flat = (yy[sel] - g * HALF_Y) * NX + xx[sel]  # [0, CORE_COLS)
            # dedupe duplicate cells, keep the LAST occurrence
            if len(flat):
                u_rev, first_rev = np.unique(flat[::-1], return_index=True)
                keep = len(flat) - 1 - first_rev
                sel, flat = sel[keep], flat[keep]
            slab = flat // SLAB
            within = flat % SLAB
            win = within // W
            loc = within % W
            # slot space: per window, both slabs share K_SLOTS slots;
            # chunk t covers slots [K*t, K*(t+1))
            order = np.argsort(win, kind="stable")
            sel, slab, win, loc = sel[order], slab[order], win[order], loc[order]
            kcounts = np.bincount(win, minlength=NWIN)
            starts = np.concatenate([[0], np.cumsum(kcounts)[:-1]])
            slot_within = np.arange(len(win)) - starts[win]
            cores.append((sel, slab, win, loc, slot_within))
            counts_per_core.append(kcounts)

    counts_max = np.max(np.stack(counts_per_core), axis=0)
    chunks_per_window = np.maximum(
        1, -(-counts_max // K_SLOTS)).astype(np.int64)
    nwt = int(chunks_per_window.sum())
    entry0 = np.concatenate([[0], np.cumsum(chunks_per_window)[:-1]])

    in_maps = []
    for (sel, slab, win, loc, slot_within) in cores:
        chunk = slot_within // K_SLOTS
        slot = (slot_within % K_SLOTS).astype(np.int64)
        entry = entry0[win] + chunk
        wt = np.zeros((nwt, K_SLOTS, 128), dtype=np.float16)
        idxc = np.full((nwt, K_SLOTS), -1.0, dtype=np.float32)
        if len(sel):
            for a in range(SLABS):
                m = slab == a
                wt[entry[m], slot[m], 64 * a : 64 * a + 64] = vf16[sel[m]]
            idxc[entry, slot] = loc.astype(np.float32)
        w_dev = np.ascontiguousarray(
            wt.transpose(1, 0, 2).reshape(K_SLOTS, nwt * 128))
        idx_dev = np.ascontiguousarray(idxc.T)
        in_maps.append({"w": w_dev, "idx": idx_dev})

    return in_maps, tuple(int(c) for c in chunks_per_window), nwt


def _run(voxel_features, coords, trace=False):
    from concourse.bass_utils import run_bass_kernel_spmd

    in_maps, chunks, nwt = _host_pack(voxel_features, coords)
    key = chunks
    if key not in _cache:
        _cache[key] = _build_program(chunks, nwt)
    nc = _cache[key]

    res = run_bass_kernel_spmd(nc, in_maps, core_ids=list(range(N_CORES)),
                               trace=trace)
    out = np.zeros((B, C, NY, NX), dtype=np.float32)
    for k in range(N_CORES):
        b, g = divmod(k, 2)
        core_out = res.results[k]["out"].reshape(-1)
        _unscramble(core_out,
                    out[b, :, g * HALF_Y : (g + 1) * HALF_Y, :])
    return out, res


def kernel(voxel_features, coords, batch_size=B):
    assert int(batch_size) == B
    out, _ = _run(voxel_features, coords, trace=False)
    return out
